# revision 1
# baseline (speedup 1.0000x reference)
"""BitNet FFN Trainium2 kernel (8-core SPMD, data-parallel over tokens).

Math (forward values of the STE reference):
  wq(w)  = clip(round(w/s), -1, 1) * s,  s = mean(|w|) + EPS        (ternary)
  xq(x)  = round(x/sx) * sx,  sx = max(absmax_row(x), EPS)/127      (int8 range)
  gate = sigmoid(xq @ wq_g.T); up = xq @ wq_u.T; h = gate*up
  out  = hq(h) @ wq_d.T

Strategy: every matmul runs in bf16 with fp32 PSUM accumulation on exact
integers (|int| <= 127 activations, ternary weights, partial sums < 2^24),
so the integer matmuls are exact; all scales are folded in fp32 outside the
matmuls. Tokens are sharded 8 ways (1024/core); each core streams the full
weights once. The only collective is a 16-byte AllReduce for the three
global weight-scale sums.
"""

import sys

sys.path.insert(0, "/opt/trn_rl_repo")

import numpy as np

import concourse.tile as tile
from concourse import bacc, mybir

F32 = mybir.dt.float32
BF16 = mybir.dt.bfloat16
ADD = mybir.AluOpType.add
SUB = mybir.AluOpType.subtract
MULT = mybir.AluOpType.mult
MAX = mybir.AluOpType.max
AXX = mybir.AxisListType.X
AFT = mybir.ActivationFunctionType

EPS = 1e-5
CR = 12582912.0  # 1.5*2^23: fp32 RNE round-to-integer magic constant
ALPHA = 1.0986122886681098  # atanh(0.5)/0.5 : tanh(ALPHA*0.5) == 0.5
P = 128


def build_program(T, DM, FF, ncores, ff_sh, dm_sh):
    """Build the per-core SPMD program.

    T: tokens per core; DM: d_model; FF: d_ff; ff_sh/dm_sh: rows of the
    per-core weight-scale shards (w_gate/w_up shard rows, w_down shard rows).
    """
    assert T % P == 0 and DM % P == 0 and FF % 1024 == 0
    MT = T // P              # token tiles
    KD = DM // P             # d_model k-blocks
    NG = FF // 1024          # phase-1 ff groups (8 strips each)
    K3 = FF // P             # phase-3 ff k-blocks
    MD = DM // P             # output dm blocks
    TN = min(512, T)         # moving free dim (tokens) per matmul
    NT3 = T // TN            # phase-3 token chunks
    WPC = min(2048, DM)      # scale-pass piece width for g/u
    WPC3 = min(2048, FF)     # scale-pass piece width for wd

    nc = bacc.Bacc(
        "TRN2",
        target_bir_lowering=False,
        debug=False,
        enable_asserts=False,
        num_devices=ncores,
    )

    x_d = nc.dram_tensor("x", [T, DM], F32, kind="ExternalInput")
    wg_d = nc.dram_tensor("wg", [FF, DM], F32, kind="ExternalInput")
    wu_d = nc.dram_tensor("wu", [FF, DM], F32, kind="ExternalInput")
    wd_d = nc.dram_tensor("wd", [DM, FF], F32, kind="ExternalInput")
    wgs_d = nc.dram_tensor("wg_sh", [ff_sh, DM], F32, kind="ExternalInput")
    wus_d = nc.dram_tensor("wu_sh", [ff_sh, DM], F32, kind="ExternalInput")
    wds_d = nc.dram_tensor("wd_sh", [dm_sh, FF], F32, kind="ExternalInput")
    out_d = nc.dram_tensor("out_t", [DM, T], F32, kind="ExternalOutput")

    NW = float(FF * DM)  # elements per weight matrix (all three equal)

    with tile.TileContext(nc, num_cores=ncores) as tc:
        import contextlib

        with contextlib.ExitStack() as outer:
            dram = outer.enter_context(tc.tile_pool(name="dram", bufs=1, space="DRAM"))
            psum = outer.enter_context(tc.tile_pool(name="psum", bufs=8, space="PSUM"))
            tiny = outer.enter_context(tc.tile_pool(name="tiny", bufs=1))

            hp_d = dram.tile([T, FF], F32)       # h' = sigmoid(G)*U_int
            shs_d = dram.tile([1, T], F32)       # per-token output scale row
            cc_in = dram.tile([1, 4], F32)
            cc_out = dram.tile([1, 4], F32)

            # persistent small tiles
            ones_col = tiny.tile([P, 1], F32)
            nc.vector.memset(ones_col, 1.0)
            ones_row = tiny.tile([1, P], F32)
            nc.vector.memset(ones_row, 1.0)
            sb_scales = tiny.tile([P, 8], F32)   # bcast: bg,bu,bd,swg,swu,swd
            sx_all = tiny.tile([P, MT], F32)     # per-token x scale (col=token tile)
            rx_all = tiny.tile([P, MT], F32)
            sxg_all = tiny.tile([P, MT], F32)    # sx*swg (sigmoid input scale)
            sxu_all = tiny.tile([P, MT], F32)    # sx*swu
            rph_all = tiny.tile([P, MT], F32)    # s_xu/s_h (h' quant scale)
            shd_all = tiny.tile([P, MT], F32)    # s_h*s_wd (output scale)
            accs = tiny.tile([P, MT, 2 * NG], F32)  # h' absmax partials

            # ---------------- S0: global weight scales ----------------
            with tc.tile_pool(name="s0", bufs=3) as s0p, tc.tile_pool(
                name="s0t", bufs=4
            ) as s0t:
                acc3 = tiny.tile([P, 4], F32)
                nc.vector.memset(acc3, 0.0)
                shard_specs = [
                    (wgs_d, 0, ff_sh, DM, WPC),
                    (wus_d, 1, ff_sh, DM, WPC),
                    (wds_d, 2, dm_sh, FF, WPC3),
                ]
                for src, col, rows, cols, pw in shard_specs:
                    for r0 in range(0, rows, P):
                        pr = min(P, rows - r0)
                        for c0 in range(0, cols, pw):
                            t_in = s0p.tile([P, pw], F32, name="s0raw")
                            nc.sync.dma_start(
                                t_in[:pr], src[r0 : r0 + pr, c0 : c0 + pw]
                            )
                            t_abs = s0p.tile([P, pw], F32, name="s0abs")
                            t_sum = s0t.tile([P, 1], F32, name="s0sum")
                            nc.scalar.activation(
                                out=t_abs[:pr],
                                in_=t_in[:pr],
                                func=AFT.Abs,
                                accum_out=t_sum[:pr],
                            )
                            nc.vector.tensor_tensor(
                                out=acc3[:pr, col : col + 1],
                                in0=acc3[:pr, col : col + 1],
                                in1=t_sum[:pr],
                                op=ADD,
                            )
                ps_s = psum.tile([P, 512], F32, name="ps_main")
                nc.tensor.matmul(
                    ps_s[:4, :1], acc3[:, :4], ones_col, start=True, stop=True
                )
                sb_s = s0t.tile([4, 1], F32, name="sb_s")
                nc.vector.tensor_copy(sb_s, ps_s[:4, :1])
                nc.sync.dma_start(cc_in[0, :4], sb_s[:, 0])
                nc.gpsimd.collective_compute(
                    "AllReduce",
                    ADD,
                    replica_groups=[list(range(ncores))],
                    ins=[cc_in[:].opt()],
                    outs=[cc_out[:].opt()],
                )
                sums_row = s0t.tile([1, 4], F32, name="sums_row")
                nc.sync.dma_start(sums_row, cc_out[:])
                sw_row = s0t.tile([1, 4], F32, name="sw_row")
                nc.vector.tensor_scalar(
                    out=sw_row, in0=sums_row, scalar1=1.0 / NW, scalar2=EPS,
                    op0=MULT, op1=ADD,
                )
                beta_row = s0t.tile([1, 4], F32, name="beta_row")
                nc.vector.reciprocal(beta_row, sw_row)
                row8 = s0t.tile([1, 8], F32, name="row8")
                nc.vector.tensor_scalar(
                    out=row8[:, 0:4], in0=beta_row, scalar1=ALPHA, scalar2=None,
                    op0=MULT, op1=mybir.AluOpType.bypass,
                )
                nc.vector.tensor_copy(row8[:, 4:8], sw_row)
                ps_b = psum.tile([P, 512], F32, name="ps_main")
                nc.tensor.matmul(
                    ps_b[:, :8], ones_row, row8, start=True, stop=True
                )
                nc.vector.tensor_copy(sb_scales, ps_b[:, :8])

            # ---------------- phase 0/1: x-quant + gate/up + h' ----------------
            with contextlib.ExitStack() as ph1:
                xqt_p = ph1.enter_context(tc.tile_pool(name="xqt", bufs=1))

                xqt = xqt_p.tile([P, KD, T], BF16)  # XqT: [dm-part, k, token]

                # x quantization (per token-tile) in its own pool scope
                with tc.tile_pool(name="xw", bufs=3) as xw_p:
                    for m in range(MT):
                        xt = xw_p.tile([P, DM], F32, name="xt")
                        nc.gpsimd.dma_start(xt, x_d[m * P : (m + 1) * P, :])
                        amax = xw_p.tile([P, 1], F32, name="amax")
                        nc.vector.tensor_reduce(
                            amax, xt, axis=AXX, op=MAX, apply_absolute_value=True
                        )
                        nc.vector.tensor_scalar(
                            out=sx_all[:, m : m + 1], in0=amax, scalar1=EPS,
                            scalar2=1.0 / 127.0, op0=MAX, op1=MULT,
                        )
                        nc.vector.reciprocal(
                            rx_all[:, m : m + 1], sx_all[:, m : m + 1]
                        )
                        nc.vector.tensor_tensor(
                            out=sxg_all[:, m : m + 1], in0=sx_all[:, m : m + 1],
                            in1=sb_scales[:, 4:5], op=MULT,
                        )
                        nc.vector.tensor_tensor(
                            out=sxu_all[:, m : m + 1], in0=sx_all[:, m : m + 1],
                            in1=sb_scales[:, 5:6], op=MULT,
                        )
                        xr = xw_p.tile([P, DM], F32, name="xr")
                        nc.vector.tensor_scalar(
                            out=xr, in0=xt, scalar1=rx_all[:, m : m + 1], scalar2=CR,
                            op0=MULT, op1=ADD,
                        )
                        xq = xw_p.tile([P, DM], BF16, name="xq")
                        nc.vector.tensor_scalar(
                            out=xq, in0=xr, scalar1=CR, scalar2=None,
                            op0=SUB, op1=mybir.AluOpType.bypass,
                        )
                        nc.sync.dma_start_transpose(
                            xqt[:, :, m * P : (m + 1) * P], xq
                        )

                wraw_p = ph1.enter_context(tc.tile_pool(name="wraw", bufs=3))
                wtern_p = ph1.enter_context(tc.tile_pool(name="wtern", bufs=3))
                wchunk_p = ph1.enter_context(tc.tile_pool(name="wchunk", bufs=6))
                gate_p = ph1.enter_context(
                    tc.tile_pool(name="gate", bufs=4)
                )
                hpr_p = ph1.enter_context(tc.tile_pool(name="hpr", bufs=2))
                sc_p = ph1.enter_context(tc.tile_pool(name="scp", bufs=2))

                # Merged gate+up pass per 512-ff group (4 strips each).
                # Ternary chunks are strip-major [P, strip(4), k(KD), 128] so
                # each strip transpose lands contiguous. One LDWEIGHTS (xqT
                # tile) feeds the G and U matmuls; 2 PSUM banks per token
                # tile so four token tiles pipeline.
                def produce_chunk(eng, wsrc, beta_col, ng):
                    chunk = wchunk_p.tile([P, 4, KD, P], BF16, name="wchunk")
                    for s4 in range(4):
                        r0 = (ng * 4 + s4) * P
                        raw = wraw_p.tile([P, DM], F32, name="wraw")
                        nc.gpsimd.dma_start(raw, wsrc[r0 : r0 + P, :])
                        nc.scalar.activation(
                            out=raw, in_=raw, func=AFT.Tanh,
                            scale=sb_scales[:, beta_col : beta_col + 1],
                        )
                        tern = wtern_p.tile([P, DM], BF16, name="wtern")
                        nc.vector.tensor_scalar(
                            out=tern, in0=raw, scalar1=CR, scalar2=CR,
                            op0=ADD, op1=SUB,
                        )
                        eng.dma_start_transpose(
                            chunk[:, s4 : s4 + 1, :, :], tern
                        )
                    return chunk

                NG5 = FF // 512
                for ng in range(NG5):
                    chunk_g = produce_chunk(nc.sync, wg_d, 0, ng)
                    chunk_u = produce_chunk(nc.sync, wu_d, 1, ng)
                    for m in range(MT):
                        psg = psum.tile([P, 512], F32, name="ps_main")
                        psu = psum.tile([P, 512], F32, name="ps_main")
                        for k in range(KD):
                            lhsT = xqt[:, k, m * P : (m + 1) * P]
                            st, sp = (k == 0), (k == KD - 1)
                            nc.tensor.matmul(
                                psg, lhsT, chunk_g[:, :, k, :], start=st, stop=sp
                            )
                            nc.tensor.matmul(
                                psu, lhsT, chunk_u[:, :, k, :], start=st, stop=sp
                            )
                        gt = gate_p.tile([P, 512], F32, name="gate_t")
                        nc.scalar.activation(
                            out=gt, in_=psg, func=AFT.Sigmoid,
                            scale=sxg_all[:, m : m + 1],
                        )
                        hp = hpr_p.tile([P, 512], F32, name="hp")
                        nc.vector.tensor_tensor(out=hp, in0=gt, in1=psu, op=MULT)
                        nc.vector.tensor_reduce(
                            accs[:, m, ng : ng + 1], hp, axis=AXX,
                            op=MAX, apply_absolute_value=True,
                        )
                        nc.scalar.dma_start(
                            hp_d[m * P : (m + 1) * P, ng * 512 : (ng + 1) * 512],
                            hp,
                        )

                # h scales per token tile
                for m in range(MT):
                    am = sc_p.tile([P, 1], F32, name="am")
                    nc.vector.tensor_reduce(
                        am, accs[:, m, :], axis=AXX, op=MAX
                    )
                    nc.vector.tensor_tensor(
                        out=am, in0=am, in1=sxu_all[:, m : m + 1], op=MULT
                    )
                    sh = sc_p.tile([P, 1], F32, name="sh")
                    nc.vector.tensor_scalar(
                        out=sh, in0=am, scalar1=EPS, scalar2=1.0 / 127.0,
                        op0=MAX, op1=MULT,
                    )
                    rs = sc_p.tile([P, 1], F32, name="rs")
                    nc.vector.reciprocal(rs, sh)
                    nc.vector.tensor_tensor(
                        out=rph_all[:, m : m + 1], in0=rs,
                        in1=sxu_all[:, m : m + 1], op=MULT,
                    )
                    nc.vector.tensor_tensor(
                        out=shd_all[:, m : m + 1], in0=sh,
                        in1=sb_scales[:, 6:7], op=MULT,
                    )
                    nc.sync.dma_start(
                        shs_d[0, m * P : (m + 1) * P], shd_all[:, m : m + 1]
                    )

            # ---------------- phase 2/3: quantize h' + down projection ----------------
            with contextlib.ExitStack() as ph23:
                hqtb_p = ph23.enter_context(tc.tile_pool(name="hqtb", bufs=1))
                # hqt: [ff-in-block, ff-block k, token] — transposed quantized h
                hqt = hqtb_p.tile([P, K3, T], BF16)

                # S5: quantize h' into hqt, ff-column-major so phase-3 matmuls
                # can consume early k columns while later ones still quantize
                with tc.tile_pool(name="s5", bufs=6) as s5p:
                    PW5 = min(2048, FF)
                    for c0 in range(0, FF, PW5):
                        for m in range(MT):
                            hpt = s5p.tile([P, PW5], F32, name="hpt")
                            nc.gpsimd.dma_start(
                                hpt, hp_d[m * P : (m + 1) * P, c0 : c0 + PW5]
                            )
                            nc.vector.tensor_scalar(
                                out=hpt, in0=hpt, scalar1=rph_all[:, m : m + 1],
                                scalar2=CR, op0=MULT, op1=ADD,
                            )
                            hqq = s5p.tile([P, PW5], BF16, name="hqq")
                            nc.vector.tensor_scalar(
                                out=hqq, in0=hpt, scalar1=CR, scalar2=None,
                                op0=SUB, op1=mybir.AluOpType.bypass,
                            )
                            nc.sync.dma_start_transpose(
                                hqt[
                                    :,
                                    c0 // P : (c0 + PW5) // P,
                                    m * P : (m + 1) * P,
                                ],
                                hqq,
                            )

                shs_p = ph23.enter_context(tc.tile_pool(name="shsp", bufs=1))
                wdr_p = ph23.enter_context(tc.tile_pool(name="wdr", bufs=2))
                wdtern_p = ph23.enter_context(tc.tile_pool(name="wdtn", bufs=1))
                wdt_p = ph23.enter_context(tc.tile_pool(name="wdtg", bufs=3))
                fin_p = ph23.enter_context(tc.tile_pool(name="finp", bufs=2))

                shs_row = shs_p.tile([1, T], F32, name="shs_row")
                nc.sync.dma_start(shs_row, shs_d[:])
                shs_bc = shs_p.tile([P, T], F32, name="shs_bc")
                for t in range(NT3):
                    ps_bc = psum.tile([P, 512], F32, name="ps_main")
                    nc.tensor.matmul(
                        ps_bc[:, :TN], ones_row,
                        shs_row[:, t * TN : (t + 1) * TN], start=True, stop=True,
                    )
                    nc.vector.tensor_copy(
                        shs_bc[:, t * TN : (t + 1) * TN], ps_bc[:, :TN]
                    )

                # fused: ternarize+transpose w_down per output dm-block,
                # full-k PSUM accumulation; emitted inside the s5 scope so
                # wd production and early matmuls overlap quantization
                KH = K3 // 2  # k-blocks per wdtg half-tile
                for md in range(MD):
                    halves = []
                    for h in range(2):
                        wdtg = wdt_p.tile([P, KH, P], BF16, name="wdtg")
                        halves.append(wdtg)
                        base = h * (FF // 2)
                        PW3 = min(2048, FF // 2)
                        for c0 in range(0, FF // 2, PW3):
                            raw = wdr_p.tile([P, PW3], F32, name="wdraw")
                            nc.gpsimd.dma_start(
                                raw,
                                wd_d[
                                    md * P : (md + 1) * P,
                                    base + c0 : base + c0 + PW3,
                                ],
                            )
                            nc.scalar.activation(
                                out=raw, in_=raw, func=AFT.Tanh,
                                scale=sb_scales[:, 2:3],
                            )
                            ternd = wdtern_p.tile([P, PW3], BF16, name="wdtern")
                            nc.vector.tensor_scalar(
                                out=ternd, in0=raw, scalar1=CR, scalar2=CR,
                                op0=ADD, op1=SUB,
                            )
                            nc.sync.dma_start_transpose(
                                wdtg[:, c0 // P : (c0 + PW3) // P, :], ternd
                            )
                    pss = [
                        psum.tile([P, 512], F32, name="ps_main")
                        for _ in range(NT3)
                    ]
                    for k in range(K3):
                        lhsT = halves[k // KH][:, k % KH, :]
                        for t in range(NT3):
                            nc.tensor.matmul(
                                pss[t][:, :TN],
                                lhsT,
                                hqt[:, k, t * TN : (t + 1) * TN],
                                start=(k == 0),
                                stop=(k == K3 - 1),
                            )
                    for t in range(NT3):
                        ot = fin_p.tile([P, TN], F32, name="ot")
                        nc.vector.tensor_tensor(
                            out=ot, in0=pss[t][:, :TN],
                            in1=shs_bc[:, t * TN : (t + 1) * TN], op=MULT,
                        )
                        nc.scalar.dma_start(
                            out_d[md * P : (md + 1) * P, t * TN : (t + 1) * TN],
                            ot,
                        )

    nc.compile()
    return nc


_CACHE = {}
TRACE = False  # set True (e.g. from test.py) to capture an NTFF profile
LAST_RESULTS = None


def _get_program(T, DM, FF, ncores, ff_sh, dm_sh):
    key = (T, DM, FF, ncores, ff_sh, dm_sh)
    if key not in _CACHE:
        _CACHE[key] = build_program(T, DM, FF, ncores, ff_sh, dm_sh)
    return _CACHE[key]


def kernel(x, w_gate, w_up, w_down):
    from concourse.bass_utils import run_bass_kernel_spmd

    x = np.asarray(x, dtype=np.float32)
    w_gate = np.ascontiguousarray(np.asarray(w_gate, dtype=np.float32))
    w_up = np.ascontiguousarray(np.asarray(w_up, dtype=np.float32))
    w_down = np.ascontiguousarray(np.asarray(w_down, dtype=np.float32))

    B, S, DM = x.shape
    FF = w_gate.shape[0]
    NCORES = 8
    NTOK = B * S
    T = NTOK // NCORES
    ff_sh = FF // NCORES
    dm_sh = DM // NCORES

    xf = np.ascontiguousarray(x.reshape(NTOK, DM))
    nc = _get_program(T, DM, FF, NCORES, ff_sh, dm_sh)

    in_maps = []
    for c in range(NCORES):
        in_maps.append(
            {
                "x": np.ascontiguousarray(xf[c * T : (c + 1) * T]),
                "wg": w_gate,
                "wu": w_up,
                "wd": w_down,
                "wg_sh": np.ascontiguousarray(w_gate[c * ff_sh : (c + 1) * ff_sh]),
                "wu_sh": np.ascontiguousarray(w_up[c * ff_sh : (c + 1) * ff_sh]),
                "wd_sh": np.ascontiguousarray(w_down[c * dm_sh : (c + 1) * dm_sh]),
            }
        )

    res = run_bass_kernel_spmd(
        nc, in_maps, core_ids=list(range(NCORES)), trace=TRACE
    )
    global LAST_RESULTS
    LAST_RESULTS = res
    out = np.empty((NTOK, DM), dtype=np.float32)
    for c in range(NCORES):
        out[c * T : (c + 1) * T] = res.results[c]["out_t"].T
    return out.reshape(B, S, DM)



# revision 19
# speedup vs baseline: 1.3576x; 1.3576x over previous
"""BitNet FFN Trainium2 kernel (8-core SPMD).

Math (forward values of the STE reference):
  wq(w)  = clip(round(w/s), -1, 1) * s,  s = mean(|w|) + EPS        (ternary)
  xq(x)  = round(x/sx) * sx,  sx = max(absmax_row(x), EPS)/127      (int8 range)
  gate = sigmoid(xq @ wq_g.T); up = xq @ wq_u.T; h = gate*up
  out  = hq(h) @ wq_d.T

Design (v2):
  - Data-parallel over tokens (1024 tok/core), but weight ternarization is
    SHARDED: each core ternarizes 1/8 of each weight matrix (host passes the
    shard pre-transposed into matmul layout), then two AllGathers distribute
    the ternary bf16/fp16 weights to all cores.  This removes the 8x
    redundant weight DMA + tanh/round work the v1 kernel did.
  - Phase 1 computes G/U with the TERNARY WEIGHT as the stationary operand,
    so h' lands in PSUM already transposed ([ff, tok]).  h' = sigmoid(G*sxg)
    * U is written to a resident SBUF tile in fp16 -- no DRAM round trip.
  - Quantization of h' to int8-valued fp16 happens IN PLACE once the global
    per-token absmax is known; phase-3 matmuls chase the quantizer per
    k-block so the PE bubble at the boundary is tiny.
  - All matmuls run on exact integers (|int|<=127 activations, ternary
    weights, fp32 PSUM accumulation), scales are folded in fp32 outside.
    The only approximation is storing h' in fp16 before quantization.
"""

import sys

sys.path.insert(0, "/opt/trn_rl_repo")

import contextlib

import numpy as np

import concourse.tile as tile
from concourse import bacc, mybir
from concourse.masks import make_identity

F32 = mybir.dt.float32
BF16 = mybir.dt.bfloat16
FP16 = mybir.dt.float16
ADD = mybir.AluOpType.add
SUB = mybir.AluOpType.subtract
MULT = mybir.AluOpType.mult
MAX = mybir.AluOpType.max
BYP = mybir.AluOpType.bypass
AXX = mybir.AxisListType.X
AFT = mybir.ActivationFunctionType

EPS = 1e-5
CR = 12582912.0  # 1.5*2^23: fp32 RNE round-to-integer magic constant
ALPHA = 1.0986122886681098  # atanh(0.5)/0.5 : tanh(ALPHA*0.5) == 0.5
P = 128


def build_program(T, DM, FF, ncores):
    """Per-core SPMD program. T tokens/core; full DM/FF; ff sharded /ncores."""
    KD = DM // P           # d_model contraction blocks
    FB = FF // P           # ff 128-blocks
    MT = T // P            # token tiles
    SH = FF // ncores      # ff rows per shard
    SB = SH // P           # ff blocks per shard
    TN = min(512, T)       # token chunk per matmul
    NTC = T // TN          # token chunks
    CW = P                 # ff columns per phase-1 weight fetch
    MD = DM // P           # output dm blocks
    MQ = 4 if MD % 4 == 0 else 1   # dm blocks per phase-3 quad
    NW = float(FF * DM)
    assert T % P == 0 and DM % P == 0 and FF % (P * ncores) == 0
    assert FB % 4 == 0

    nc = bacc.Bacc(
        "TRN2",
        target_bir_lowering=False,
        debug=False,
        enable_asserts=False,
        num_devices=ncores,
    )

    x_d = nc.dram_tensor("x", [T, DM], F32, kind="ExternalInput")
    wgt_d = nc.dram_tensor("wgt_sh", [DM, SH], F32, kind="ExternalInput")
    wut_d = nc.dram_tensor("wut_sh", [DM, SH], F32, kind="ExternalInput")
    wdt_d = nc.dram_tensor("wdt_sh", [SH, DM], F32, kind="ExternalInput")
    out_d = nc.dram_tensor("out_t", [DM, T], F32, kind="ExternalOutput")

    RG = [list(range(ncores))]

    with tile.TileContext(nc, num_cores=ncores) as tc:
        with contextlib.ExitStack() as S:
            dram = S.enter_context(tc.tile_pool(name="dram", bufs=1, space="DRAM"))
            psum = S.enter_context(tc.tile_pool(name="psum", bufs=8, space="PSUM"))
            tiny = S.enter_context(tc.tile_pool(name="tiny", bufs=1))

            # DRAM scratch: AllGather bounce buffers.
            gin_gu = dram.tile([2 * DM, SH], BF16)
            gout_gu = dram.tile([ncores * 2 * DM, SH], BF16, addr_space="Shared")
            gin_d = dram.tile([SH, DM], FP16)
            gout_d = dram.tile([FF, DM], FP16, addr_space="Shared")
            ar_in = dram.tile([1, 4], F32)
            ar_out = dram.tile([1, 4], F32)
            rows_d = dram.tile([2, T], F32)   # rowify bounce (sx, amax)

            # persistent small tiles
            ones_row = tiny.tile([1, P], F32)
            nc.vector.memset(ones_row, 1.0)
            ones_col = tiny.tile([P, 1], F32)
            nc.vector.memset(ones_col, 1.0)
            ident = tiny.tile([P, P], F32)
            make_identity(nc, ident)

            sw_cells = tiny.tile([1, 3], F32)    # s_wg, s_wu, s_wd
            beta_bc = tiny.tile([P, 3], F32)     # ALPHA/s_w per partition
            sxu_row = tiny.tile([1, T], F32)     # sx*s_wu (survives to phase 2)
            sx_cols = tiny.tile([P, MT], F32)
            am_cols = tiny.tile([P, MT], F32)

            def rowify(cols, row, slot):
                """cols [P, MT] (col m = tokens m*P..(m+1)*P) -> row [1, T]."""
                pst = psum.tile([P, 512], F32, name="ps")
                nc.tensor.transpose(pst[:MT, :P], cols[:, :MT], ident)
                sb_t = tiny.tile([MT, P], F32, name="rowify_t")
                nc.vector.tensor_copy(sb_t, pst[:MT, :P])
                nc.sync.dma_start(rows_d[slot, :], sb_t[:, :])
                nc.sync.dma_start(row, rows_d[slot : slot + 1, :])

            def bcast(row, out_bc):
                """row [1, T] -> out_bc [P, T] (same value down partitions)."""
                for t in range(NTC):
                    psb = psum.tile([P, 512], F32, name="ps")
                    nc.tensor.matmul(
                        psb[:, :TN], ones_row, row[:, t * TN : (t + 1) * TN],
                        start=True, stop=True,
                    )
                    nc.vector.tensor_copy(out_bc[:, t * TN : (t + 1) * TN], psb[:, :TN])

            # ============ S0: sharded weight-scale scan + tiny AllReduce ======
            with contextlib.ExitStack() as pre:
                whold_p = pre.enter_context(tc.tile_pool(name="whold", bufs=1))
                wg_hold = whold_p.tile([P, KD, SH], F32)
                wu_hold = whold_p.tile([P, KD, SH], F32)
                nc.sync.dma_start(
                    wg_hold, wgt_d[:, :].rearrange("(i p) f -> p i f", p=P)
                )
                nc.sync.dma_start(
                    wu_hold, wut_d[:, :].rearrange("(i p) f -> p i f", p=P)
                )
                acc = tiny.tile([P, 3], F32)
                nc.vector.tensor_reduce(
                    acc[:, 0:1], wg_hold.rearrange("p i f -> p (i f)"),
                    axis=AXX, op=ADD, apply_absolute_value=True,
                )
                nc.vector.tensor_reduce(
                    acc[:, 1:2], wu_hold.rearrange("p i f -> p (i f)"),
                    axis=AXX, op=ADD, apply_absolute_value=True,
                )
                # wd scan: streaming (tiles are reloaded later for ternarize)
                with tc.tile_pool(name="wdscan", bufs=3) as wds_p:
                    nc.vector.memset(acc[:, 2:3], 0.0)
                    for i in range(SB):
                        wdt = wds_p.tile([P, DM], F32, name="wdscan_t")
                        nc.scalar.dma_start(wdt, wdt_d[i * P : (i + 1) * P, :])
                        part = wds_p.tile([P, 1], F32, name="wdscan_s")
                        nc.vector.tensor_reduce(
                            part, wdt, axis=AXX, op=ADD, apply_absolute_value=True
                        )
                        nc.vector.tensor_tensor(
                            out=acc[:, 2:3], in0=acc[:, 2:3], in1=part, op=ADD
                        )

                pss = psum.tile([P, 512], F32, name="ps")
                nc.tensor.matmul(pss[:3, :1], acc[:, :3], ones_col, start=True, stop=True)
                sums_sb = tiny.tile([3, 1], F32)
                nc.vector.tensor_copy(sums_sb, pss[:3, :1])
                nc.sync.dma_start(ar_in[0, :3], sums_sb[:, 0])
                nc.gpsimd.collective_compute(
                    "AllReduce",
                    ADD,
                    replica_groups=RG,
                    ins=[ar_in[:, :3].opt()],
                    outs=[ar_out[:, :3].opt()],
                )
                arld = tiny.tile([1, 3], F32)
                nc.sync.dma_start(arld, ar_out[:, :3])
                nc.vector.tensor_scalar(
                    out=sw_cells, in0=arld, scalar1=1.0 / NW, scalar2=EPS,
                    op0=MULT, op1=ADD,
                )
                binv = tiny.tile([1, 3], F32)
                nc.vector.reciprocal(binv, sw_cells)
                nc.vector.tensor_scalar(
                    out=binv, in0=binv, scalar1=ALPHA, scalar2=None, op0=MULT, op1=BYP
                )
                psb3 = psum.tile([P, 512], F32, name="ps")
                nc.tensor.matmul(psb3[:, :3], ones_row, binv, start=True, stop=True)
                nc.vector.tensor_copy(beta_bc, psb3[:, :3])

                # ---- ternarize wg/wu shards from held tiles -> gin_gu -> AG
                with tc.tile_pool(name="tern", bufs=2) as tern_p:
                    for src_hold, base, col in ((wg_hold, 0, 0), (wu_hold, DM, 1)):
                        for i in range(KD):
                            tf = tern_p.tile([P, SH], F32, name="tern_f")
                            nc.scalar.activation(
                                out=tf, in_=src_hold[:, i, :], func=AFT.Tanh,
                                scale=beta_bc[:, col : col + 1],
                            )
                            tq = tern_p.tile([P, SH], BF16, name="tern_q")
                            nc.vector.tensor_scalar(
                                out=tq, in0=tf, scalar1=CR, scalar2=CR,
                                op0=ADD, op1=SUB,
                            )
                            nc.sync.dma_start(
                                gin_gu[base + i * P : base + (i + 1) * P, :], tq
                            )
                nc.gpsimd.collective_compute(
                    "AllGather",
                    BYP,
                    replica_groups=RG,
                    ins=[gin_gu[:].opt()],
                    outs=[gout_gu[:].opt()],
                )

            # ============ persistent big tiles across phases 1..3 =============
            big_p = S.enter_context(tc.tile_pool(name="big", bufs=1))
            hpT = big_p.tile([P, FB, T], FP16)    # h' (later: quantized h)
            mxa = big_p.tile([P, T], F32)         # running absmax of |h'|
            nc.vector.memset(mxa, 0.0)

            with contextlib.ExitStack() as mid:
                mid_p = mid.enter_context(tc.tile_pool(name="mid", bufs=1))
                xqt = mid_p.tile([P, KD, T], BF16)   # x quantized, transposed
                sxg_bc = mid_p.tile([P, T], FP16)
                sx_row = mid_p.tile([1, T], F32)
                sxg_row = mid_p.tile([1, T], F32)

                # ======== phase 0: x quantization (overlaps the AllGather) ====
                with tc.tile_pool(name="xw", bufs=1) as xw_p:
                    for m in range(MT):
                        xt = xw_p.tile([P, DM], F32, name="xt")
                        nc.scalar.dma_start(xt, x_d[m * P : (m + 1) * P, :])
                        amax = xw_p.tile([P, 1], F32, name="amax")
                        nc.vector.tensor_reduce(
                            amax, xt, axis=AXX, op=MAX, apply_absolute_value=True
                        )
                        nc.vector.tensor_scalar(
                            out=sx_cols[:, m : m + 1], in0=amax, scalar1=EPS,
                            scalar2=1.0 / 127.0, op0=MAX, op1=MULT,
                        )
                        rx = xw_p.tile([P, 1], F32, name="rx")
                        nc.vector.reciprocal(rx, sx_cols[:, m : m + 1])
                        nc.vector.tensor_scalar(
                            out=xt, in0=xt, scalar1=rx, scalar2=CR, op0=MULT, op1=ADD
                        )
                        xq = xw_p.tile([P, DM], BF16, name="xq")
                        nc.vector.tensor_scalar(
                            out=xq, in0=xt, scalar1=CR, scalar2=None, op0=SUB, op1=BYP
                        )
                        nc.scalar.dma_start_transpose(
                            xqt[:, :, m * P : (m + 1) * P], xq
                        )
                    rowify(sx_cols, sx_row, 0)
                    nc.vector.tensor_scalar(
                        out=sxg_row, in0=sx_row, scalar1=sw_cells[:, 0:1],
                        scalar2=None, op0=MULT, op1=BYP,
                    )
                    nc.vector.tensor_scalar(
                        out=sxu_row, in0=sx_row, scalar1=sw_cells[:, 1:2],
                        scalar2=None, op0=MULT, op1=BYP,
                    )
                    bcast(sxg_row, sxg_bc)

                # ---- ternarize wd shard (reload) -> gin_d -> AG (late) -------
                with tc.tile_pool(name="ternd", bufs=1) as td_p:
                    for i in range(SB):
                        wdt = td_p.tile([P, DM], F32, name="ternd_in")
                        nc.scalar.dma_start(wdt, wdt_d[i * P : (i + 1) * P, :])
                        nc.scalar.activation(
                            out=wdt, in_=wdt, func=AFT.Tanh, scale=beta_bc[:, 2:3]
                        )
                        tq = td_p.tile([P, DM], FP16, name="ternd_q")
                        nc.vector.tensor_scalar(
                            out=tq, in0=wdt, scalar1=CR, scalar2=CR, op0=ADD, op1=SUB
                        )
                        nc.scalar.dma_start(gin_d[i * P : (i + 1) * P, :], tq)
                nc.gpsimd.collective_compute(
                    "AllGather",
                    BYP,
                    replica_groups=RG,
                    ins=[gin_d[:].opt()],
                    outs=[gout_d[:].opt()],
                )

                # ======== phase 1: gate/up matmuls, h' -> hpT (fp16) ==========
                with contextlib.ExitStack() as ph1:
                    wch_p = ph1.enter_context(tc.tile_pool(name="wch", bufs=2))
                    gt_p = ph1.enter_context(tc.tile_pool(name="gt", bufs=1))
                    for c in range(ncores):
                        for ch in range(SH // CW):
                            # one fetch holds the G and U k-stacks for this
                            # 128-wide ff block: [P, 2*KD, CW]
                            guch = wch_p.tile([P, 2 * KD, CW], BF16, name="guch")
                            base = c * 2 * DM
                            cs = slice(ch * CW, (ch + 1) * CW)
                            nc.sync.dma_start(
                                guch,
                                gout_gu[base : base + 2 * DM, cs].rearrange(
                                    "(i p) f -> p i f", p=P
                                ),
                            )
                            f = c * SB + ch
                            psG = [
                                psum.tile([P, 512], F32, name="ps")
                                for _ in range(NTC)
                            ]
                            psU = [
                                psum.tile([P, 512], F32, name="ps")
                                for _ in range(NTC)
                            ]
                            for ps_list, koff in ((psG, 0), (psU, KD)):
                                for k in range(KD):
                                    lhsT = guch[:, koff + k, :]
                                    st, sp = (k == 0), (k == KD - 1)
                                    for t in range(NTC):
                                        nc.tensor.matmul(
                                            ps_list[t][:, :TN],
                                            lhsT,
                                            xqt[:, k, t * TN : (t + 1) * TN],
                                            start=st,
                                            stop=sp,
                                        )
                            gt = gt_p.tile([P, T], F32, name="gt")
                            for t in range(NTC):
                                ts_ = slice(t * TN, (t + 1) * TN)
                                nc.vector.tensor_tensor(
                                    out=gt[:, ts_], in0=psG[t][:, :TN],
                                    in1=sxg_bc[:, ts_], op=MULT,
                                )
                            nc.scalar.activation(out=gt, in_=gt, func=AFT.Sigmoid)
                            for t in range(NTC):
                                ts_ = slice(t * TN, (t + 1) * TN)
                                nc.vector.tensor_tensor(
                                    out=hpT[:, f, ts_], in0=gt[:, ts_],
                                    in1=psU[t][:, :TN], op=MULT,
                                )
                            # gt is dead after the h' write: reuse it as the
                            # |h'| scratch for the running absmax
                            nc.scalar.activation(
                                out=gt, in_=hpT[:, f, :], func=AFT.Abs
                            )
                            nc.vector.tensor_tensor(
                                out=mxa, in0=mxa, in1=gt, op=MAX
                            )

            # ============ phase 2: global h scales + in-place quantize ========
            with contextlib.ExitStack() as ph2:
                ph2_p = ph2.enter_context(tc.tile_pool(name="ph2", bufs=1))
                # per-token absmax over partitions: PE transpose + free-axis max
                for m in range(MT):
                    pst = psum.tile([P, 512], F32, name="ps")
                    nc.tensor.transpose(
                        pst[:P, :P], mxa[:, m * P : (m + 1) * P], ident
                    )
                    nc.vector.tensor_reduce(
                        am_cols[:, m : m + 1], pst[:P, :P], axis=AXX, op=MAX
                    )
                amax_row = ph2_p.tile([1, T], F32)
                sh_row = ph2_p.tile([1, T], F32)
                rph_row = ph2_p.tile([1, T], F32)
                shd_row = ph2_p.tile([1, T], F32)
                rowify(am_cols, amax_row, 1)
                nc.vector.tensor_tensor(
                    out=amax_row, in0=amax_row, in1=sxu_row, op=MULT
                )
                nc.vector.tensor_scalar(
                    out=sh_row, in0=amax_row, scalar1=EPS, scalar2=1.0 / 127.0,
                    op0=MAX, op1=MULT,
                )
                nc.vector.reciprocal(rph_row, sh_row)
                nc.vector.tensor_tensor(
                    out=rph_row, in0=rph_row, in1=sxu_row, op=MULT
                )
                nc.vector.tensor_scalar(
                    out=shd_row, in0=sh_row, scalar1=sw_cells[:, 2:3],
                    scalar2=None, op0=MULT, op1=BYP,
                )
                rph_bc = big_p.tile([P, T], F32)
                shd_bc = big_p.tile([P, T], F32)
                bcast(rph_row, rph_bc)
                bcast(shd_row, shd_bc)
                qtmp = ph2_p.tile([P, T], F32)
                for k in range(FB):
                    nc.vector.tensor_tensor(
                        out=qtmp, in0=hpT[:, k, :], in1=rph_bc, op=MULT
                    )
                    nc.vector.tensor_scalar(
                        out=hpT[:, k, :], in0=qtmp, scalar1=CR, scalar2=CR,
                        op0=ADD, op1=SUB,
                    )

                # ======== phase 3: down projection ============================
                with contextlib.ExitStack() as ph3:
                    wd3_p = ph3.enter_context(tc.tile_pool(name="wd3", bufs=3))
                    fin_p = ph3.enter_context(tc.tile_pool(name="fin", bufs=2))
                    DMQ = MQ * P
                    for q in range(MD // MQ):
                        ps3 = [
                            psum.tile([P, 512], F32, name="ps")
                            for _ in range(MQ * NTC)
                        ]
                        for g in range(FB // 4):
                            wtile = wd3_p.tile([P, 4, DMQ], FP16, name="wd3t")
                            nc.sync.dma_start(
                                wtile,
                                gout_d[
                                    g * 4 * P : (g + 1) * 4 * P,
                                    q * DMQ : (q + 1) * DMQ,
                                ].rearrange("(i p) f -> p i f", p=P),
                            )
                            for kb in range(4):
                                k = g * 4 + kb
                                st, sp = (k == 0), (k == FB - 1)
                                for mi in range(MQ):
                                    lhsT = wtile[:, kb, mi * P : (mi + 1) * P]
                                    for t in range(NTC):
                                        nc.tensor.matmul(
                                            ps3[mi * NTC + t][:, :TN],
                                            lhsT,
                                            hpT[:, k, t * TN : (t + 1) * TN],
                                            start=st,
                                            stop=sp,
                                        )
                        for mi in range(MQ):
                            md = q * MQ + mi
                            ot = fin_p.tile([P, T], F32, name="ot")
                            for t in range(NTC):
                                ts_ = slice(t * TN, (t + 1) * TN)
                                nc.vector.tensor_tensor(
                                    out=ot[:, ts_], in0=ps3[mi * NTC + t][:, :TN],
                                    in1=shd_bc[:, ts_], op=MULT,
                                )
                            nc.scalar.dma_start(
                                out_d[md * P : (md + 1) * P, :], ot
                            )

    nc.compile()
    return nc


_CACHE = {}
TRACE = False
LAST_RESULTS = None


def _get_program(T, DM, FF, ncores):
    key = (T, DM, FF, ncores)
    if key not in _CACHE:
        _CACHE[key] = build_program(T, DM, FF, ncores)
    return _CACHE[key]


def kernel(x, w_gate, w_up, w_down):
    from concourse.bass_utils import run_bass_kernel_spmd

    x = np.asarray(x, dtype=np.float32)
    w_gate = np.asarray(w_gate, dtype=np.float32)
    w_up = np.asarray(w_up, dtype=np.float32)
    w_down = np.asarray(w_down, dtype=np.float32)

    B, S, DM = x.shape
    FF = w_gate.shape[0]
    NCORES = 8
    NTOK = B * S
    T = NTOK // NCORES
    SH = FF // NCORES

    xf = np.ascontiguousarray(x.reshape(NTOK, DM))
    nc = _get_program(T, DM, FF, NCORES)

    in_maps = []
    for c in range(NCORES):
        sl = slice(c * SH, (c + 1) * SH)
        in_maps.append(
            {
                "x": np.ascontiguousarray(xf[c * T : (c + 1) * T]),
                "wgt_sh": np.ascontiguousarray(w_gate[sl].T),
                "wut_sh": np.ascontiguousarray(w_up[sl].T),
                "wdt_sh": np.ascontiguousarray(w_down[:, sl].T),
            }
        )

    res = run_bass_kernel_spmd(
        nc, in_maps, core_ids=list(range(NCORES)), trace=TRACE
    )
    global LAST_RESULTS
    LAST_RESULTS = res
    out = np.empty((NTOK, DM), dtype=np.float32)
    for c in range(NCORES):
        out[c * T : (c + 1) * T] = res.results[c]["out_t"].T
    return out.reshape(B, S, DM)
